# revision 3
# baseline (speedup 1.0000x reference)
"""2-layer GCN (PyG GCNConv) on 8 Trainium2 NeuronCores — v2.

Strategy (node-parallel, per sharding hint), rebuilt around three wins
over the v1 baseline:
  1. bf16 everywhere on the hot path (PE matmuls 4x faster than fp32,
     gather DMA bytes halved).
  2. Batched `dma_gather` (one instruction per ~50 chunks) instead of
     one `indirect_dma_start` per 128-edge chunk — SWDGE descriptor
     generation drops from ~1.9ms to ~0.1ms of Q7 time.
  3. The symmetric norm deg^-1/2[src]*deg^-1/2[dst] is separable:
     dis[src] is folded into the all-gathered table rows (ACT scale on
     the phase-1/3 PSUM->SBUF copy), dis[dst] into the output copy, and
     the bias is pre-divided via a rank-1 (dis_inv x b) matmul. The
     per-chunk selection matrices become pure one-hots, built for a
     whole gather-group in ONE broadcast-AP tensor_tensor(is_equal)
     instead of one tensor_scalar per chunk.

Layout:
  - Nodes range-sharded: R=6250/core, padded to RP=6272 (49 tiles x 128).
    Padded global node id: srcp = core*RP + local. Table rows 0..50175.
  - Edges (incl. self-loops) bucketed by (dst tile, src-half) where
    lo: srcp < 32768, hi: srcp >= 32768 (dma_gather indices are int16;
    hi gathers use a +32768 base AP). Within a bucket, edges pack into
    128-wide chunks; chunk counts are max'd over cores (shared SPMD
    program), with dstl=255 padding (one-hot row of zeros).
  - Gather groups of TG=4 dst tiles share one lo + one hi dma_gather and
    one one-hot build.
  - Phase 1: xw1 = x @ W1 (bf16), rows scaled by dis -> AllGather.
  - Phase 2: per chunk matmul S_onehot.T @ G accumulating in PSUM;
    +dis_inv x b1 rank-1; ACT Relu with scale=dis -> h1 (node-major),
    PE-transposed into h1T for phase 3.
  - Phase 3: hw2 = h1 @ W2, scaled by dis -> AllGather.
  - Phase 4: same aggregation at F2=128; +dis_inv x b2; ACT Copy with
    scale=dis -> fp32 out rows.
"""

import sys

for p in ("/opt/trn_rl_repo",):
    if p not in sys.path:
        sys.path.insert(0, p)

import numpy as np
import ml_dtypes

import concourse.bass as bass
import concourse.bacc as bacc
import concourse.mybir as mybir
import concourse.tile as tile
from concourse import bass_utils
from concourse.masks import make_identity

P = 128
NCORES = 8
SPLIT = 32768
TG = 4  # dst tiles per gather group
PAD_DST = 255.0  # dstl value marking a padded edge slot
# dma_gather caps: with single_packet=True the HW packet holds <=64
# descriptors per engine -> num_idxs <= 1024 (8 chunks) per instruction,
# and crashes the device above that. single_packet=False removes the cap
# but its descriptor generation is ~20x slower (measured 47us/6k-idx op),
# so capped single-packet ops win.
GCAP = 8
SP_GATHER = True
S_DMAJOR = True  # d-major one-hot build (DVE 2x path); False = legacy bcast


# ----------------------------------------------------------------------------
# Host-side preprocessing
# ----------------------------------------------------------------------------

def _preprocess(x, edge_index):
    N = x.shape[0]
    R = N // NCORES
    assert R * NCORES == N
    ntiles = (R + P - 1) // P
    RP = ntiles * P

    src = np.asarray(edge_index[0], np.int64)
    dst = np.asarray(edge_index[1], np.int64)
    loops = np.arange(N, dtype=np.int64)
    src = np.concatenate([src, loops])
    dst = np.concatenate([dst, loops])

    deg = np.bincount(dst, minlength=N).astype(np.float64)
    dis = np.where(deg > 0, 1.0 / np.sqrt(deg), 0.0)  # [N] f64

    c_src = src // R
    srcp = c_src * RP + (src - c_src * R)  # padded global row
    c_dst = dst // R
    dloc = dst - c_dst * R
    t_of = dloc // P
    dl_of = dloc - t_of * P
    half = (srcp >= SPLIT).astype(np.int64)

    key = (c_dst * ntiles + t_of) * 2 + half
    counts = np.bincount(key, minlength=NCORES * ntiles * 2)
    counts = counts.reshape(NCORES, ntiles, 2)
    maxc = counts.max(axis=0)  # [ntiles, 2]
    chunks = ((maxc + P - 1) // P).astype(np.int64)  # [ntiles, 2]

    # group layout: [lo(t0)..lo(t3) | hi(t0)..hi(t3)] chunk-slots
    groups = []
    slot_base = np.zeros((ntiles, 2), np.int64)
    base = 0
    for g0 in range(0, ntiles, TG):
        ts = list(range(g0, min(g0 + TG, ntiles)))
        gbase = base
        for t in ts:
            slot_base[t, 0] = base
            base += chunks[t, 0]
        nlo = base - gbase
        for t in ts:
            slot_base[t, 1] = base
            base += chunks[t, 1]
        nhi = base - gbase - nlo
        tslots = {}
        for t in ts:
            tslots[t] = (
                list(range(slot_base[t, 0], slot_base[t, 0] + chunks[t, 0])),
                list(range(slot_base[t, 1], slot_base[t, 1] + chunks[t, 1])),
            )
        groups.append(dict(tiles=ts, base=gbase, nlo=int(nlo), nhi=int(nhi),
                           tslots=tslots))
    NS = int(base)

    # per-core packed arrays
    per_core = []
    for c in range(NCORES):
        m = c_dst == c
        t_c = t_of[m]
        h_c = half[m]
        sp_c = srcp[m]
        dl_c = dl_of[m]
        # sort by (tile, half, src) — src-ordering within a bucket gives the
        # gather's HBM reads some locality for free
        order = np.lexsort((sp_c, h_c, t_c))
        t_c, h_c, sp_c, dl_c = t_c[order], h_c[order], sp_c[order], dl_c[order]
        sec = t_c * 2 + h_c  # section id, nondecreasing
        cnt = np.bincount(sec, minlength=ntiles * 2)
        starts = np.cumsum(cnt) - cnt
        pos = np.arange(len(sec)) - starts[sec]
        sbase = slot_base.reshape(-1)  # [ntiles*2]
        flat = (sbase[sec] * P + pos).astype(np.int64)

        A_idx = np.zeros(NS * P, np.int32)
        A_dl = np.full(NS * P, PAD_DST, np.float32)
        A_idx[flat] = (sp_c - h_c * SPLIT).astype(np.int32)
        A_dl[flat] = dl_c

        # [16, NS*8] wrap block replicated to all 8 Q7-core partition stripes
        idx16 = np.tile(A_idx.astype(np.int16).reshape(NS * 8, 16).T, (8, 1))
        dstl = np.ascontiguousarray(
            A_dl.reshape(NS, P).T).astype(ml_dtypes.bfloat16)

        NSG = max(g["nlo"] + g["nhi"] for g in groups)
        nd = np.arange(RP)
        gl = c * R + nd
        valid = nd < R
        disp_core = np.where(valid, dis[np.minimum(gl, N - 1)], 0.0)
        disinv_core = np.where(valid & (disp_core > 0), 1.0 /
                               np.maximum(disp_core, 1e-30), 0.0)
        disp = np.ascontiguousarray(
            disp_core.reshape(ntiles, P).T).astype(np.float32)  # [P, ntiles]
        disinv = disinv_core.reshape(1, RP).astype(ml_dtypes.bfloat16)

        xs = np.zeros((RP, x.shape[1]), np.float32)
        xs[:R] = x[c * R:(c + 1) * R]
        xT = np.ascontiguousarray(xs.T).astype(ml_dtypes.bfloat16)

        # iota_rep[p, d*NSG + s] = d  (d-major one-hot comparison operand;
        # physically replicated across partitions so all APs stay packed)
        iotar = np.broadcast_to(
            np.repeat(np.arange(P, dtype=np.float32), NSG)[None, :],
            (P, P * NSG)).astype(ml_dtypes.bfloat16)

        per_core.append(dict(idx16=idx16, dstl=dstl, disp=disp,
                             disinv=disinv, xT=xT, iotar=iotar))

    layout = dict(ntiles=ntiles, RP=RP, NS=NS, groups=groups,
                  chunks=chunks, slot_base=slot_base)
    return per_core, layout


# ----------------------------------------------------------------------------
# Device kernel
# ----------------------------------------------------------------------------

def build_nc(layout, F0, F1, F2):
    f32 = mybir.dt.float32
    bf16 = mybir.dt.bfloat16
    i16 = mybir.dt.int16
    i32 = mybir.dt.int32

    ntiles = layout["ntiles"]
    RP = layout["RP"]
    NS = layout["NS"]
    groups = layout["groups"]
    NPAD = NCORES * RP
    K0 = F0 // P
    K2 = F1 // P
    H1 = F1 // P
    NSG = max(g["nlo"] + g["nhi"] for g in groups)

    nc = bacc.Bacc("TRN2", target_bir_lowering=False, debug=False,
                   num_devices=NCORES, num_swdge_queues=4)

    xT_d = nc.dram_tensor("xT", [F0, RP], bf16, kind="ExternalInput").ap()
    iotar_d = nc.dram_tensor("iotar", [P, P * NSG], bf16,
                             kind="ExternalInput").ap()
    idx_d = nc.dram_tensor("idx16", [P, NS * 8], i16, kind="ExternalInput").ap()
    dstl_d = nc.dram_tensor("dstl", [P, NS], bf16, kind="ExternalInput").ap()
    disp_d = nc.dram_tensor("disp", [P, ntiles], f32, kind="ExternalInput").ap()
    dinv_d = nc.dram_tensor("disinv", [1, RP], bf16, kind="ExternalInput").ap()
    W1_d = nc.dram_tensor("W1", [F0, F1], bf16, kind="ExternalInput").ap()
    b1_d = nc.dram_tensor("b1", [1, F1], bf16, kind="ExternalInput").ap()
    W2_d = nc.dram_tensor("W2", [F1, F2], bf16, kind="ExternalInput").ap()
    b2_d = nc.dram_tensor("b2", [1, F2], bf16, kind="ExternalInput").ap()
    out_d = nc.dram_tensor("out", [RP, F2], f32, kind="ExternalOutput").ap()

    rg = [list(range(NCORES))]

    qrr = [0]  # round-robin SWDGE queue cursor (4 queues -> parallel
    #            descriptor generation on the Q7 cluster)

    def emit_gathers(G, table_ap, idx_sb, gb, nslots, loc0, Fw):
        """Gather `nslots` chunk-slots (idx cols starting at gb) from
        table_ap into G at local slot offset loc0, capped at GCAP
        chunks per dma_gather instruction."""
        off = 0
        while off < nslots:
            cnt = min(GCAP, nslots - off)
            nc.gpsimd.dma_gather(
                G[:, (loc0 + off) * Fw:(loc0 + off + cnt) * Fw].rearrange(
                    "p (s e) -> p s e", s=cnt),
                table_ap,
                idx_sb[:, (gb + off) * 8:(gb + off + cnt) * 8],
                cnt * P, cnt * P, Fw, single_packet=SP_GATHER,
                queue_num=qrr[0])
            qrr[0] = (qrr[0] + 1) % 4
            off += cnt

    def mkap(base, off, dims):
        """Manual AP: base tile's partition dim + custom free-axis layout."""
        return bass.AP(base.tensor, int(base.offset) + int(off),
                       [[int(v) for v in base.ap[0]]]
                       + [[int(v) for v in d] for d in dims])

    with tile.TileContext(nc) as tc:
        with (
            tc.tile_pool(name="dram", bufs=1, space="DRAM") as dram,
            tc.tile_pool(name="const", bufs=1) as const,
        ):
            ag1_in = dram.tile([RP, F1], bf16)
            ag1_out = dram.tile([NPAD, F1], bf16, addr_space="Shared")
            ag2_in = dram.tile([RP, F2], bf16)
            ag2_out = dram.tile([NPAD, F2], bf16, addr_space="Shared")

            idx_sb = const.tile([P, NS * 8], i16)
            nc.sync.dma_start(out=idx_sb[:], in_=idx_d[:])
            dstl_sb = const.tile([P, NS], bf16)
            nc.sync.dma_start(out=dstl_sb[:], in_=dstl_d[:])
            disp_sb = const.tile([P, ntiles], f32)
            nc.sync.dma_start(out=disp_sb[:], in_=disp_d[:])
            dinv_sb = const.tile([1, RP], bf16)
            nc.sync.dma_start(out=dinv_sb[:], in_=dinv_d[:])
            w2_sb = const.tile([P, K2 * F2], bf16)
            nc.sync.dma_start(
                out=w2_sb[:].rearrange("p (k f) -> p k f", k=K2),
                in_=W2_d.rearrange("(k p) f -> p k f", p=P))
            b1_sb = const.tile([1, F1], bf16)
            nc.sync.dma_start(out=b1_sb[:], in_=b1_d[:])
            b2_sb = const.tile([1, F2], bf16)
            nc.sync.dma_start(out=b2_sb[:], in_=b2_d[:])

            iotar_sb = const.tile([P, P * NSG], bf16)
            nc.sync.dma_start(out=iotar_sb[:], in_=iotar_d[:])
            ident = const.tile([P, P], bf16)
            make_identity(nc, ident[:])

            h1T = const.tile([P, H1 * RP], bf16)

            def build_onehot(S, gb, nsg):
                """S one-hot for a group's chunk-slots [gb, gb+nsg)."""
                if S_DMAJOR:
                    # S[p, d*NSG + s] = (dstl[p, gb+s] == d); packed last dims
                    nc.vector.tensor_tensor(
                        out=mkap(S[:], 0, [[NSG, P], [1, nsg]]),
                        in0=mkap(dstl_sb[:], gb, [[0, P], [1, nsg]]),
                        in1=mkap(iotar_sb[:], 0, [[NSG, P], [1, nsg]]),
                        op=mybir.AluOpType.is_equal)
                else:
                    # s-major: S[p, s*P + d] = (dstl[p, gb+s] == d)
                    nc.vector.tensor_tensor(
                        out=mkap(S[:], 0, [[P, nsg], [1, P]]),
                        in0=mkap(dstl_sb[:], gb, [[1, nsg], [0, P]]),
                        in1=mkap(iotar_sb[:], 0, [[0, nsg], [NSG, P]]),
                        op=mybir.AluOpType.is_equal)

            def slhs(S, sl):
                """lhsT AP for chunk-slot sl of a group's S tile."""
                if S_DMAJOR:
                    return mkap(S[:], sl, [[NSG, P]])
                return S[:, sl * P:(sl + 1) * P]

            # ---------------- phase 1: table1 = dis * (x @ W1) -------------
            with (
                tc.tile_pool(name="p1x", bufs=1) as p1x,
                tc.tile_pool(name="p1o", bufs=4) as p1o,
                tc.tile_pool(name="p1ps", bufs=4, space="PSUM") as p1ps,
            ):
                w1_sb = p1x.tile([P, K0 * F1], bf16)
                nc.sync.dma_start(
                    out=w1_sb[:].rearrange("p (k f) -> p k f", k=K0),
                    in_=W1_d.rearrange("(k p) f -> p k f", p=P))
                xt_sb = p1x.tile([P, K0 * RP], bf16)
                nc.sync.dma_start(
                    out=xt_sb[:].rearrange("p (k r) -> p k r", k=K0),
                    in_=xT_d.rearrange("(k p) r -> p k r", p=P))
                for m in range(ntiles):
                    ps = p1ps.tile([P, F1], f32)
                    for k in range(K0):
                        nc.tensor.matmul(
                            out=ps[:],
                            lhsT=xt_sb[:, k * RP + m * P: k * RP + (m + 1) * P],
                            rhs=w1_sb[:, k * F1:(k + 1) * F1],
                            start=(k == 0), stop=(k == K0 - 1))
                    os = p1o.tile([P, F1], bf16)
                    nc.scalar.activation(
                        out=os[:], in_=ps[:],
                        func=mybir.ActivationFunctionType.Copy,
                        scale=disp_sb[:, m:m + 1])
                    nc.sync.dma_start(out=ag1_in[m * P:(m + 1) * P, :],
                                      in_=os[:])

            nc.gpsimd.collective_compute(
                "AllGather", mybir.AluOpType.bypass, replica_groups=rg,
                ins=[ag1_in[:].opt()], outs=[ag1_out[:].opt()])

            # ---------------- phase 2: h1 = relu(dis*agg + b1) -------------
            with (
                tc.tile_pool(name="p2g", bufs=2) as p2g,
                tc.tile_pool(name="p2s", bufs=2) as p2s,
                tc.tile_pool(name="p2h", bufs=4) as p2h,
                tc.tile_pool(name="p2ps", bufs=6, space="PSUM") as p2ps,
                tc.tile_pool(name="p2pt", bufs=2, space="PSUM") as p2pt,
            ):
                for g in groups:
                    nlo, nhi = g["nlo"], g["nhi"]
                    nsg = nlo + nhi
                    gb = g["base"]
                    G = p2g.tile([P, NSG * F1], bf16, tag="G")
                    if nlo:
                        emit_gathers(G, ag1_out[:], idx_sb, gb, nlo, 0, F1)
                    if nhi:
                        emit_gathers(G, ag1_out[SPLIT:, :], idx_sb,
                                     gb + nlo, nhi, nlo, F1)
                    # d-major one-hot: S[p, d*NSG + s] = (dstl[p, gb+s] == d)
                    # all operands keep packed 2-byte last dims -> DVE 2x mode
                    S = p2s.tile([P, NSG * P], bf16, tag="S")
                    build_onehot(S, gb, nsg)
                    pss = {}
                    for t in g["tiles"]:
                        pss[t] = p2ps.tile([P, F1], f32, tag="ps",
                                           name=f"ps2_{t}")
                    for hsel in (0, 1):
                        for t in g["tiles"]:
                            lo_s, hi_s = g["tslots"][t]
                            slots = lo_s if hsel == 0 else hi_s
                            first_overall = (hsel == 0 and len(lo_s) > 0) or \
                                            (hsel == 1 and len(lo_s) == 0)
                            for i, s in enumerate(slots):
                                sl = s - gb
                                nc.tensor.matmul(
                                    out=pss[t][:],
                                    lhsT=slhs(S, sl),
                                    rhs=G[:, sl * F1:(sl + 1) * F1],
                                    start=(first_overall and i == 0),
                                    stop=False)
                    for t in g["tiles"]:
                        nc.tensor.matmul(
                            out=pss[t][:],
                            lhsT=dinv_sb[:, t * P:(t + 1) * P],
                            rhs=b1_sb[:], start=False, stop=True)
                        hm = p2h.tile([P, F1], bf16, tag="hm")
                        nc.scalar.activation(
                            out=hm[:], in_=pss[t][:],
                            func=mybir.ActivationFunctionType.Relu,
                            scale=disp_sb[:, t:t + 1])
                        pt = p2pt.tile([P, H1 * P], bf16, tag="pt")
                        for h in range(H1):
                            nc.tensor.transpose(
                                out=pt[:, h * P:(h + 1) * P],
                                in_=hm[:, h * P:(h + 1) * P],
                                identity=ident[:])
                            nc.vector.tensor_copy(
                                out=h1T[:, h * RP + t * P: h * RP + (t + 1) * P],
                                in_=pt[:, h * P:(h + 1) * P])

            # ---------------- phase 3: table2 = dis * (h1 @ W2) ------------
            with (
                tc.tile_pool(name="p3o", bufs=4) as p3o,
                tc.tile_pool(name="p3ps", bufs=4, space="PSUM") as p3ps,
            ):
                for m in range(ntiles):
                    ps = p3ps.tile([P, F2], f32)
                    for k in range(K2):
                        nc.tensor.matmul(
                            out=ps[:],
                            lhsT=h1T[:, k * RP + m * P: k * RP + (m + 1) * P],
                            rhs=w2_sb[:, k * F2:(k + 1) * F2],
                            start=(k == 0), stop=(k == K2 - 1))
                    os = p3o.tile([P, F2], bf16)
                    nc.scalar.activation(
                        out=os[:], in_=ps[:],
                        func=mybir.ActivationFunctionType.Copy,
                        scale=disp_sb[:, m:m + 1])
                    nc.sync.dma_start(out=ag2_in[m * P:(m + 1) * P, :],
                                      in_=os[:])

            nc.gpsimd.collective_compute(
                "AllGather", mybir.AluOpType.bypass, replica_groups=rg,
                ins=[ag2_in[:].opt()], outs=[ag2_out[:].opt()])

            # ---------------- phase 4: out = dis*agg + b2 ------------------
            with (
                tc.tile_pool(name="p4g", bufs=2) as p4g,
                tc.tile_pool(name="p4s", bufs=2) as p4s,
                tc.tile_pool(name="p4o", bufs=4) as p4o,
                tc.tile_pool(name="p4ps", bufs=8, space="PSUM") as p4ps,
            ):
                for g in groups:
                    nlo, nhi = g["nlo"], g["nhi"]
                    nsg = nlo + nhi
                    gb = g["base"]
                    G = p4g.tile([P, NSG * F2], bf16, tag="G4")
                    if nlo:
                        emit_gathers(G, ag2_out[:], idx_sb, gb, nlo, 0, F2)
                    if nhi:
                        emit_gathers(G, ag2_out[SPLIT:, :], idx_sb,
                                     gb + nlo, nhi, nlo, F2)
                    S = p4s.tile([P, NSG * P], bf16, tag="S4")
                    build_onehot(S, gb, nsg)
                    pss = {}
                    for t in g["tiles"]:
                        pss[t] = p4ps.tile([P, F2], f32, tag="ps4",
                                           name=f"ps4_{t}")
                    for hsel in (0, 1):
                        for t in g["tiles"]:
                            lo_s, hi_s = g["tslots"][t]
                            slots = lo_s if hsel == 0 else hi_s
                            first_overall = (hsel == 0 and len(lo_s) > 0) or \
                                            (hsel == 1 and len(lo_s) == 0)
                            for i, s in enumerate(slots):
                                sl = s - gb
                                nc.tensor.matmul(
                                    out=pss[t][:],
                                    lhsT=slhs(S, sl),
                                    rhs=G[:, sl * F2:(sl + 1) * F2],
                                    start=(first_overall and i == 0),
                                    stop=False)
                    for t in g["tiles"]:
                        nc.tensor.matmul(
                            out=pss[t][:],
                            lhsT=dinv_sb[:, t * P:(t + 1) * P],
                            rhs=b2_sb[:], start=False, stop=True)
                        os = p4o.tile([P, F2], f32, tag="o4")
                        nc.scalar.activation(
                            out=os[:], in_=pss[t][:],
                            func=mybir.ActivationFunctionType.Copy,
                            scale=disp_sb[:, t:t + 1])
                        nc.sync.dma_start(
                            out=out_d[t * P:(t + 1) * P, :], in_=os[:])

    nc.compile()
    return nc


# ----------------------------------------------------------------------------
# Public entry point
# ----------------------------------------------------------------------------

LAST_EXEC_NS = None
LAST_RESULTS = None


def kernel(x, edge_index, W1, b1, W2, b2, _trace=False, _tmpdir=None):
    global LAST_EXEC_NS, LAST_RESULTS
    x = np.asarray(x, np.float32)
    edge_index = np.asarray(edge_index)
    W1 = np.asarray(W1, np.float32)
    b1 = np.asarray(b1, np.float32)
    W2 = np.asarray(W2, np.float32)
    b2 = np.asarray(b2, np.float32)
    N, F0 = x.shape
    F1 = W1.shape[1]
    F2 = W2.shape[1]

    per_core, layout = _preprocess(x, edge_index)
    nc = build_nc(layout, F0, F1, F2)

    W1b = W1.astype(ml_dtypes.bfloat16)
    W2b = W2.astype(ml_dtypes.bfloat16)
    b1b = b1.reshape(1, -1).astype(ml_dtypes.bfloat16)
    b2b = b2.reshape(1, -1).astype(ml_dtypes.bfloat16)

    in_maps = []
    for c in range(NCORES):
        pc = per_core[c]
        in_maps.append({
            "xT": pc["xT"], "idx16": pc["idx16"], "dstl": pc["dstl"],
            "disp": pc["disp"], "disinv": pc["disinv"], "iotar": pc["iotar"],
            "W1": W1b, "b1": b1b, "W2": W2b, "b2": b2b,
        })

    res = bass_utils.run_bass_kernel_spmd(
        nc, in_maps, core_ids=list(range(NCORES)), trace=_trace,
        tmpdir=_tmpdir)
    LAST_EXEC_NS = res.exec_time_ns
    LAST_RESULTS = res
    R = N // NCORES
    out = np.concatenate(
        [res.results[c]["out"][:R] for c in range(NCORES)], axis=0)
    return out.astype(np.float32)
